# revision 2
# baseline (speedup 1.0000x reference)
"""CRF Viterbi decode (nn_CRF, B=512 T=512 O=64) on 8 Trainium2 NeuronCores.

Pure data parallel: 64 sequences per core; the (64, 64) transition matrix and
derived constants are replicated.

Forward (per step t), layout p = g*64 + b, tag j = g*32 + j_lo:
  ps[p, i]      = S0 @ x_t + S1 @ x_t (+ S0 @ m2_t + S1 @ m2_t)   (PE PSUM
                  accumulation: state_t = m2_t + x_t rebuilt in replicated
                  layout; the x folds are off the critical path)
  state_hist[0:64, t, :] = ps[0:64, :]                  (ACT copy, off-path;
                  feeds the backward with no per-step fold matmuls)
  ts[p, j_lo, i] = trans_rep + ps-broadcast             (split DVE/Pool: the
                  Pool engine covers most j_lo columns in parallel with DVE)
  m2_{t+1}      = max_i ts                              (DVE segmented reduce)

Backward runs as TWO independent 32-sequence half-chains (partitions 0:32 and
32:64) that interleave on the engines, hiding each chain's PE/sem latency
behind the other's DVE work. Per half and step t:
  gps[hs, :]  = one-hot(tag_{t+1})^T-blocks @ {tio_s, tio_c}   (2 K=32 PE mms;
                the 32x32 DVE block transpose only permutes within blocks, the
                cross table fixes the off-diagonal blocks)
  cand        = gps + state_full[hs, t, :]               (DVE STT, into SBUF)
  exact first-argmax + one-hot via the bmi trick (ties like jnp.argmax):
    negmax = -max(cand); h_any = (cand + negmax == 0)
    t1 = (h_any * ne) * (64 - j);  mi = -max(t1)  (tag = mi + 64)
    h = ((64 - j) + mi == 0); hBT = block-transpose(h)
  ne = not_end[:, t-1]: 0 at t == L poisons h to all-zero, so the gather
  collapses and cand reduces to state_{L-1}, reproducing init_tag/init_conf.
  conf: ACT Exp with bias=negmax, accum -> scoreb (reciprocal at epilogue).

Positions >= L are zeroed by the mask, matching the reference.

Hardware caveats kept from the baseline:
- matmul operands at partition base 64 crash (PE quadrant-3 bug): all
  contractions stay at base 0/32;
- start_tensor_calc=True lazily zeroes the whole per-partition PSUM bank, so
  each accumulation group has exactly one start=True mm and the rest
  accumulate in chained order.
"""
import numpy as np

_B, _T, _O = 512, 512, 64
_NCORES = 8
_BL = _B // _NCORES

_CACHE = {}

_ADVE = 19  # j_lo columns added on DVE; the Pool engine covers the rest


def _host_constants(trans):
    trans = np.ascontiguousarray(trans.astype(np.float32))
    transT = np.ascontiguousarray(trans.T)                  # [j, i]
    # trans_rep2[g*64+b, j_lo, half, i_lo]: half 0 = own g-block of i
    # (g' == g), half 1 = cross block (g' == 1-g). The forward's "own" add
    # then reads the state from the same partition (m2 + x), and only the
    # cross half needs the swap matmul.
    trans_rep = np.zeros((128, 32, 2, 32), np.float32)
    for g in range(2):
        rows = slice(g * 64, (g + 1) * 64)
        own = transT[g * 32:(g + 1) * 32, g * 32:(g + 1) * 32]
        cross = transT[g * 32:(g + 1) * 32, (1 - g) * 32:(2 - g) * 32]
        trans_rep[rows, :, 0, :] = own[None, :, :]
        trans_rep[rows, :, 1, :] = cross[None, :, :]
    trans_rep = np.ascontiguousarray(trans_rep)
    Ssw = np.zeros((128, 128), np.float32)
    for g in range(2):
        for b in range(64):
            Ssw[(1 - g) * 64 + b, g * 64 + b] = 1.0
    tio_s = np.ascontiguousarray(transT)                    # [64, 64]
    tio_c = np.ascontiguousarray(
        np.concatenate([tio_s[32:64], tio_s[0:32]], axis=0))
    iotaj = np.ascontiguousarray(
        np.broadcast_to(np.arange(64, dtype=np.float32), (64, 64)))
    eye64 = np.ascontiguousarray(np.eye(64, dtype=np.float32))
    return {
        "trans_rep": trans_rep,
        "Ssw": Ssw,
        "tio_s": tio_s,
        "tio_c": tio_c,
        "iotaj": iotaj,
        "eye64": eye64,
    }


def _host_percore(logits_c, seq_c, T):
    x2 = np.ascontiguousarray(
        logits_c.astype(np.float32)
        .reshape(_BL, T, 2, 32).transpose(2, 0, 1, 3).reshape(128, T, 32)
    )
    not_end = np.ones((_BL, T), np.float32)
    not_end[np.arange(_BL), np.maximum(seq_c - 1, 0)] = 0.0
    mask = (np.arange(T)[None, :] < seq_c[:, None]).astype(np.float32)
    return {"x2": x2, "not_end": not_end, "mask": mask}


def _in_maps(logits, transition_params, sequence_lengths, T):
    logits = np.asarray(logits, dtype=np.float32)
    trans = np.asarray(transition_params, dtype=np.float32)
    seq = np.asarray(sequence_lengths, dtype=np.int32)
    consts = _host_constants(trans)
    maps = []
    for c in range(_NCORES):
        sl = slice(c * _BL, (c + 1) * _BL)
        pc = _host_percore(logits[sl], seq[sl], T)
        m = {"x2": pc["x2"], "not_end": pc["not_end"], "mask": pc["mask"]}
        m.update(consts)
        maps.append(m)
    return maps


def _build_tile_program(tc, outs, ins, T, CT=64):
    from contextlib import ExitStack
    import concourse.bass as bass
    from concourse import mybir
    from concourse.tile import add_dep_helper

    F32 = mybir.dt.float32
    AX = mybir.AxisListType
    OP = mybir.AluOpType
    ACT = mybir.ActivationFunctionType

    nc = tc.nc
    tags_d, conf_d = outs
    (x2_d, notend_d, mask_d, transrep_d, ssw_d, tios_d, tioc_d,
     iotaj_d, eye64_d) = ins

    def bcast_mid(ap2d, n):
        assert len(ap2d.ap) == 2, ap2d.ap
        return bass.AP(tensor=ap2d.tensor, offset=ap2d.offset,
                       ap=[ap2d.ap[0], [0, n], ap2d.ap[1]])

    def chain_mms(insts):
        for a, b in zip(insts[1:], insts[:-1]):
            add_dep_helper(a.ins, b.ins, sync=False,
                           reason="psum accumulation order")
        return insts[-1]

    with ExitStack() as ctx:
        consts = ctx.enter_context(tc.tile_pool(name="consts", bufs=1))
        big = ctx.enter_context(tc.tile_pool(name="big", bufs=1))
        work = ctx.enter_context(tc.tile_pool(name="work", bufs=1))
        bwork = ctx.enter_context(tc.tile_pool(name="bwork", bufs=3))
        tspool = ctx.enter_context(tc.tile_pool(name="tspool", bufs=2))
        m2pool = ctx.enter_context(tc.tile_pool(name="m2pool", bufs=2))
        xchunks = ctx.enter_context(tc.tile_pool(name="xchunks", bufs=3))
        pstf = ctx.enter_context(
            tc.tile_pool(name="pstf", bufs=2, space="PSUM"))
        psb = ctx.enter_context(
            tc.tile_pool(name="psb", bufs=3, space="PSUM"))

        trans_rep = consts.tile([128, 32, 2, 32], F32)
        nc.sync.dma_start(trans_rep, transrep_d)
        Ssw = consts.tile([128, 128], F32)
        nc.sync.dma_start(Ssw, ssw_d)
        tio_s = consts.tile([64, 64], F32)
        nc.sync.dma_start(tio_s, tios_d)
        tio_c = consts.tile([64, 64], F32)
        nc.sync.dma_start(tio_c, tioc_d)
        iotaj = consts.tile([64, 64], F32)
        nc.sync.dma_start(iotaj, iotaj_d)
        eye64 = consts.tile([64, 64], F32)
        nc.sync.dma_start(eye64, eye64_d)
        notend = consts.tile([64, T], F32)
        nc.sync.dma_start(notend, notend_d)
        maskt = consts.tile([64, T], F32)
        nc.sync.dma_start(maskt, mask_d)

        state_full = big.tile([64, T, 64], F32)
        scoreb = big.tile([64, T], F32)
        idx8 = big.tile([64, T, 8], mybir.dt.uint16)

        # ---------------- forward ----------------
        nchunks = (T + CT - 1) // CT
        m2prev = None
        for c in range(nchunks):
            t0 = c * CT
            ct = min(CT, T - t0)
            xc = xchunks.tile([128, CT, 32], F32, tag="xc")
            nc.sync.dma_start(xc[:, :ct, :], x2_d[:, t0:t0 + ct, :])
            for tt in range(ct):
                t = t0 + tt
                pcross = pstf.tile([128, 32], F32, tag="pcross")
                j0 = nc.tensor.matmul(pcross, Ssw, xc[:, tt, :],
                                      start=True, stop=(t == 0),
                                      skip_group_check=True)
                if t > 0:
                    j1 = nc.tensor.matmul(pcross, Ssw, m2prev,
                                          start=False, stop=True,
                                          skip_group_check=True)
                    chain_mms([j0, j1])
                so = m2pool.tile([128, 32], F32, tag="so")
                if t == 0:
                    nc.vector.tensor_copy(so, xc[:, 0, :])
                else:
                    nc.vector.tensor_tensor(out=so, in0=m2prev,
                                            in1=xc[:, tt, :], op=OP.add)
                scpy = m2pool.tile([128, 32], F32, tag="scpy")
                nc.scalar.copy(scpy, pcross)
                nc.scalar.copy(state_full[0:64, t, 0:32], so[0:64, :])
                nc.scalar.copy(state_full[0:64, t, 32:64], scpy[0:64, :])
                if t < T - 1:
                    ts = tspool.tile([128, 32, 2, 32], F32, tag="ts")
                    nc.vector.tensor_tensor(
                        out=ts[:, :, 0, :], in0=trans_rep[:, :, 0, :],
                        in1=bcast_mid(so[:], 32), op=OP.add)
                    nc.vector.tensor_tensor(
                        out=ts[:, :, 1, :], in0=trans_rep[:, :, 1, :],
                        in1=bcast_mid(scpy[:], 32), op=OP.add)
                    m2 = m2pool.tile([128, 32], F32, tag="m2")
                    nc.vector.tensor_reduce(m2, ts, axis=AX.XY, op=OP.max)
                    m2prev = m2

        # ---------------- backward (two interleaved half-chains) ----------
        # cand is assembled fully in PSUM: an eye64 state prefill (start=True,
        # independent of the chain) plus the two one-hot gather matmuls.
        # argmax via InstMax/InstMaxIndex (first-occurrence ties, like
        # jnp.argmax); the sequence-end reset rides on ne multiplying the
        # one-hot, and tags are the stored uint16 indices themselves.
        HS = (slice(0, 32), slice(32, 64))
        hBT = [None, None]
        for t in range(T - 1, -1, -1):
            for h in (0, 1):
                hs = HS[h]
                gps = psb.tile([64, 64], F32, tag=f"gps{h}")
                p0 = nc.tensor.matmul(
                    gps[hs, :], eye64[:, hs], state_full[:, t, :],
                    start=True, stop=(t == T - 1), skip_group_check=True)
                mms = [p0]
                if t < T - 1:
                    if h == 0:
                        g0 = nc.tensor.matmul(
                            gps[hs, :], hBT[0][0:32, 0:32], tio_s[0:32, :],
                            start=False, stop=False, skip_group_check=True)
                        g1 = nc.tensor.matmul(
                            gps[hs, :], hBT[0][0:32, 32:64], tio_c[0:32, :],
                            start=False, stop=True, skip_group_check=True)
                    else:
                        g0 = nc.tensor.matmul(
                            gps[hs, :], hBT[1][32:64, 0:32], tio_c[32:64, :],
                            start=False, stop=False, skip_group_check=True)
                        g1 = nc.tensor.matmul(
                            gps[hs, :], hBT[1][32:64, 32:64], tio_s[32:64, :],
                            start=False, stop=True, skip_group_check=True)
                    mms += [g0, g1]
                chain_mms(mms)

                mx = bwork.tile([64, 8], F32, tag=f"mx{h}")
                nc.vector.max(mx[hs, :], gps[hs, :])
                nc.vector.max_index(idx8[hs, t, :], mx[hs, :], gps[hs, :])
                negmx = bwork.tile([64, 1], F32, tag=f"negmx{h}")
                nc.vector.tensor_scalar(out=negmx[hs, :], in0=mx[hs, 0:1],
                                        scalar1=-1.0, scalar2=None,
                                        op0=OP.mult)
                if t > 0:
                    idxf = bwork.tile([64, 1], F32, tag=f"idxf{h}")
                    nc.vector.tensor_copy(idxf[hs, :], idx8[hs, t, 0:1])
                    hh = bwork.tile([64, 64], F32, tag=f"hh{h}")
                    ne = notend[hs, t - 1:t]
                    nc.vector.tensor_scalar(out=hh[hs, :], in0=iotaj[hs, :],
                                            scalar1=idxf[hs, :], scalar2=ne,
                                            op0=OP.is_equal, op1=OP.mult)
                    hBTt = bwork.tile([64, 64], F32, tag=f"hBT{h}")
                    nc.vector.transpose(hBTt[hs, :], hh[hs, :])
                    hBT[h] = hBTt
                e = bwork.tile([64, 64], F32, tag=f"e{h}")
                nc.scalar.activation(out=e[hs, :], in_=gps[hs, :],
                                     func=ACT.Exp, bias=negmx[hs, :],
                                     scale=1.0,
                                     accum_out=scoreb[hs, t:t + 1])

        # ---------------- epilogue ----------------
        recip = work.tile([64, T], F32, tag="recip")
        nc.vector.reciprocal(recip, scoreb)
        conf = work.tile([64, T], F32, tag="conf")
        nc.vector.tensor_tensor(out=conf, in0=recip, in1=maskt, op=OP.mult)
        nc.sync.dma_start(conf_d, conf)
        tagsf = work.tile([64, T], F32, tag="tagsf")
        nc.vector.tensor_copy(tagsf, idx8[:, :, 0])
        tagsm = work.tile([64, T], F32, tag="tagsm")
        nc.vector.tensor_tensor(out=tagsm, in0=tagsf, in1=maskt, op=OP.mult)
        tagsi = work.tile([64, T], mybir.dt.int32, tag="tagsi")
        nc.vector.tensor_copy(tagsi, tagsm)
        nc.sync.dma_start(tags_d, tagsi)


def _get_compiled(T):
    key = ("nc", T)
    if key in _CACHE:
        return _CACHE[key]
    import concourse.bacc as bacc
    import concourse.tile as tile
    from concourse import mybir

    F32 = mybir.dt.float32
    I32 = mybir.dt.int32
    nc = bacc.Bacc("TRN2", target_bir_lowering=False, debug=False,
                   num_devices=_NCORES)

    ins_spec = [
        ("x2", [128, T, 32], F32),
        ("not_end", [64, T], F32),
        ("mask", [64, T], F32),
        ("trans_rep", [128, 32, 2, 32], F32),
        ("Ssw", [128, 128], F32),
        ("tio_s", [64, 64], F32),
        ("tio_c", [64, 64], F32),
        ("iotaj", [64, 64], F32),
        ("eye64", [64, 64], F32),
    ]
    ins = tuple(
        nc.dram_tensor(name, shape, dt, kind="ExternalInput").ap()
        for name, shape, dt in ins_spec
    )
    outs = (
        nc.dram_tensor("tags", [64, T], I32, kind="ExternalOutput").ap(),
        nc.dram_tensor("conf", [64, T], F32, kind="ExternalOutput").ap(),
    )

    with tile.TileContext(nc) as tc:
        _build_tile_program(tc, outs, ins, T=T)
    nc.compile()
    _CACHE[key] = nc
    return nc


def _run(logits, transition_params, sequence_lengths, trace=False):
    from concourse.bass_utils import run_bass_kernel_spmd

    T = np.asarray(logits).shape[1]
    in_maps = _in_maps(logits, transition_params, sequence_lengths, T)
    nc = _get_compiled(T)
    res = run_bass_kernel_spmd(nc, in_maps, list(range(_NCORES)),
                               trace=trace)
    tags = np.concatenate([np.asarray(res.results[c]["tags"])
                           for c in range(_NCORES)], axis=0)
    conf = np.concatenate([np.asarray(res.results[c]["conf"])
                           for c in range(_NCORES)], axis=0)
    return (tags.astype(np.int32), conf.astype(np.float32)), res


def kernel(logits, transition_params, sequence_lengths):
    (tags, conf), _ = _run(logits, transition_params, sequence_lengths)
    return tags, conf
